# revision 49
# baseline (speedup 1.0000x reference)
"""Trainium2 Bass kernel for the masked-bottleneck block (topk_masking).

Full inputs in, full outputs out. Internally shards batch (32) across 8
NeuronCores (4 images each); all parameters replicated.

Per-core layout: channels on partitions, pixels on the free dim. Each 56x56
image is processed in 7 row-tiles of 8 rows; conv2's 3x3 halo is handled by
keeping h1 in a zero-padded 58x58 layout, with a +1-row-shifted duplicate in
partitions 64..127 so two of the three ky taps share one K=128 matmul.
The mask conv rides along as a 65th conv1 output row (conv1 in exact fp32 so
the sign thresholds match the fp32 reference bit-for-bit; min |soft| is
5.8e-6, so any reduced-precision conv1 flips mask bits). Dilation is a single
K=9 matmul over 9 shifted copies of the binary mask, which simultaneously
broadcasts the (scaled) mask across partitions; masking is a vector `min`
against BIG*mask. dilate/conv2/conv3 run in bf16 (1 cyc/col on the PE, half
the SBUF/DMA bytes). f32r is NOT used: on this toolchain mixing f32r matmuls
into the program degrades the exact fp32 conv1 to f32r-level error (~5e-4),
flipping mask bits.

Structural points vs the naive version: x is triple-buffered and loaded on
the Pool SWDGE queue (never queues behind stores); the mask9 gather is 3
DMAs (one per ky covering its 3 kx shifts) instead of 9; output stores issue
from the ACT hwdge queue right after the relu that produces them.
"""

import numpy as np

import concourse.bass as bass
import concourse.tile as tile
from concourse import bacc, mybir

EPS = 1e-5
BIG = 1e30

B, CIN, H, W = 32, 256, 56, 56
NCORES = 8
BL = B // NCORES          # images per core
WID = 64                  # bottleneck width
WP = W + 2                # padded row width  (58)
NPIX = H * W              # 3136
NPAD = WP * WP            # 3364
ROWS = 8                  # output rows per tile
NT = ROWS * WP            # 464 matmul free size per tile
NC1 = ROWS * W            # 448 conv1 tile size
NTILES = H // ROWS        # 7

# sbuf buffer geometries (elements per partition)
XSZ = 1 + NPIX + 3        # x: slack elem 0, data at 1..3136, tail slack
H1SZ = 1 + NPAD + 3       # h1: slack elem 0, padded image at 1..3364
H2SZ = 1 + NPAD + 1       # h2 (65 rows: 0-63 h2m, 64 mask01)
M9SZ = H * WP             # mask9: rows p=0..55, cols q=0..57 (3248)

F32 = mybir.dt.float32


def _build_nc(mask_b_val: float, nreps: int = 1, mode: str = "bf16",
              debug_soft: bool = False,
              f32r_stages: frozenset = frozenset({"dil", "c2", "c3", "id"})):
    if mode == "f32":
        f32r_stages = frozenset()
    fast_dt = mybir.dt.bfloat16 if mode == "bf16" else mybir.dt.float32r
    def dt_of(stage):
        return fast_dt if stage in f32r_stages else mybir.dt.float32
    DT_DIL, DT_C2, DT_C3, DT_ID = (dt_of(s) for s in ("dil", "c2", "c3", "id"))
    nc = bacc.Bacc("TRN2", target_bir_lowering=False, debug=False)

    x_d = nc.declare_dram_parameter("x", [BL, CIN, H, W], F32, isOutput=False)
    w1_d = nc.declare_dram_parameter("w1aug", [CIN, WID + 1], F32, isOutput=False)
    b1_d = nc.declare_dram_parameter("b1aug", [WID + 1, 1], F32, isOutput=False)
    w2a_d = nc.declare_dram_parameter("w2k01", [128, 3 * WID], DT_C2, isOutput=False)
    w2b_d = nc.declare_dram_parameter("w2k2", [WID, 3 * WID], DT_C2, isOutput=False)
    b2_d = nc.declare_dram_parameter("b2s", [WID, 1], F32, isOutput=False)
    w3_d = nc.declare_dram_parameter("w3aug", [WID + 1, CIN], DT_C3, isOutput=False)
    dil_d = nc.declare_dram_parameter("dilw", [9, 128], DT_DIL, isOutput=False)
    out_d = nc.declare_dram_parameter("out", [BL, CIN, H, W], F32, isOutput=True)
    soft_d = (nc.declare_dram_parameter("softdbg", [BL, NPIX], F32, isOutput=True)
              if debug_soft else None)

    with tile.TileContext(nc) as tc:
        with (
            tc.tile_pool(name="consts", bufs=1) as cpool,
            tc.tile_pool(name="bigbufs", bufs=1) as bigp,
            tc.tile_pool(name="outbuf", bufs=6) as outp,
            tc.tile_pool(name="tmp2", bufs=3) as tmpp,
            tc.tile_pool(name="p1", bufs=2, space="PSUM") as p1p,
            tc.tile_pool(name="pdm", bufs=2, space="PSUM") as pdmp,
            tc.tile_pool(name="p2", bufs=2, space="PSUM") as p2p,
            tc.tile_pool(name="p3", bufs=2, space="PSUM") as p3p,
        ):
            dma = nc.sync.dma_start

            # ---- constants -------------------------------------------------
            w1_lo = cpool.tile([128, WID + 1], F32, tag="w1lo")
            w1_hi = cpool.tile([128, WID + 1], F32, tag="w1hi")
            dma(w1_lo[:], w1_d[0:128, :])
            dma(w1_hi[:], w1_d[128:256, :])
            w2a = cpool.tile([128, 3 * WID], DT_C2, tag="w2a")
            dma(w2a[:], w2a_d[:])
            w2b = cpool.tile([WID, 3 * WID], DT_C2, tag="w2b")
            dma(w2b[:], w2b_d[:])
            w3a = cpool.tile([WID + 1, CIN], DT_C3, tag="w3a")
            dma(w3a[:], w3_d[:])
            dilw = cpool.tile([9, 128], DT_DIL, tag="dilw")
            dma(dilw[:], dil_d[:])
            b1s = cpool.tile([WID + 1, 1], F32, tag="b1s")
            dma(b1s[:], b1_d[:])
            b2s = cpool.tile([WID, 1], F32, tag="b2s")
            dma(b2s[:], b2_d[:])

            # ---- persistent multi-buffered image buffers -------------------
            # x is triple-buffered: the load of image i+1 then only waits on
            # image i-2's readers, so it streams well ahead of conv1(i+1).
            x_t = [[bigp.tile([128, XSZ], F32, tag=f"x{s}c{c}", name=f"x{s}c{c}")
                    for c in (0, 1)] for s in (0, 1, 2)]
            h1_t = [bigp.tile([128, H1SZ], DT_C2, tag=f"h1{s}", name=f"h1{s}")
                    for s in (0, 1)]
            h2_t = [bigp.tile([WID + 1, H2SZ], DT_C3, tag=f"h2{s}", name=f"h2{s}")
                    for s in (0, 1)]
            m9_t = [bigp.tile([9, M9SZ], DT_DIL, tag=f"m9{s}", name=f"m9{s}")
                    for s in (0, 1)]

            # one-time zero of padding (interior is rewritten every image,
            # pads/slack must be exactly 0 for conv halos and mask taps)
            for s in (0, 1):
                for tl, sz, np_ in ((h1_t[s], H1SZ, 128), (h2_t[s], H2SZ, WID + 1)):
                    nc.vector.memset(tl[:, 0:1 + WP], 0.0)
                    nc.vector.memset(
                        bass.AP(tl.tensor, 1 + W + 1,
                                [[sz, np_], [WP, WP - 1], [1, 2]]),
                        0.0)
                    nc.vector.memset(tl[:, 1 + (WP - 1) * WP: sz], 0.0)
            for s in (0, 1, 2):
                for c in (0, 1):
                    nc.vector.memset(x_t[s][c][:, 0:1], 0.0)
                    nc.vector.memset(x_t[s][c][:, 1 + NPIX: XSZ], 0.0)

            def h1_ap(s, t):
                # interior rows 8t+1..8t+8, cols 1..56 of padded h1 (strided)
                return bass.AP(h1_t[s].tensor, 1 + (8 * t + 1) * WP + 1,
                               [[H1SZ, WID], [WP, ROWS], [1, W]])

            def h2_row64_ap(s, t):
                return bass.AP(h2_t[s].tensor, 64 * H2SZ + 1 + (8 * t + 1) * WP + 1,
                               [[H2SZ, 1], [WP, ROWS], [1, W]])

            def h2_int_ap(s, t):
                return bass.AP(h2_t[s].tensor, 1 + (8 * t + 1) * WP + 1,
                               [[H2SZ, WID], [WP, ROWS], [1, W]])

            N = BL * nreps

            def emit_load(j):
                # on the Pool SWDGE queue: never queues behind stores/gathers
                for c in (0, 1):
                    nc.gpsimd.dma_start(
                        bass.AP(x_t[j % 3][c].tensor, 1, [[XSZ, 128], [1, NPIX]]),
                        x_d[j % BL, 128 * c:128 * (c + 1)]
                        .rearrange("c h w -> c (h w)"))

            pdms = {0: [None] * NTILES, 1: [None] * NTILES}

            def emit_A_tile(bi, t):
                b, s, s3 = bi % BL, bi % 2, bi % 3
                p1 = p1p.tile([WID + 1, NC1], F32, tag="p1", name="p1")
                nc.tensor.matmul(p1[:], w1_lo[:],
                                 x_t[s3][0][:, 1 + NC1 * t: 1 + NC1 * (t + 1)],
                                 start=True, stop=False)
                nc.tensor.matmul(p1[:], w1_hi[:],
                                 x_t[s3][1][:, 1 + NC1 * t: 1 + NC1 * (t + 1)],
                                 start=False, stop=True)
                # h1 = relu(conv1 + b1) into padded layout
                nc.scalar.activation(
                    h1_ap(s, t),
                    p1[0:WID, :].rearrange("p (r w) -> p r w", w=W),
                    mybir.ActivationFunctionType.Relu, bias=b1s[:WID, 0:1])
                if soft_d is not None:
                    sb_dbg = outp.tile([1, NC1], F32, tag="sdbg", name="sdbg")
                    nc.vector.tensor_copy(sb_dbg[:], p1[WID:WID + 1, :])
                    dma(soft_d[b, NC1 * t: NC1 * (t + 1)].unsqueeze(0), sb_dbg[:])
                # mask01 = (soft + mask_b >= 0) into h2 row 64
                nc.vector.tensor_scalar(
                    h2_row64_ap(s, t),
                    p1[WID:WID + 1, :].rearrange("p (r w) -> p r w", w=W),
                    -mask_b_val, None, mybir.AluOpType.is_ge)

            def emit_gather(bi):
                # mask9 gather: 9 shifted copies of mask01 in 3 DMAs; one DMA
                # per ky covers its 3 kx shifts (dst partitions 3ky..3ky+2)
                s = bi % 2
                for ky in range(3):
                    dma(bass.AP(m9_t[s].tensor, 3 * ky * M9SZ,
                                [[M9SZ, 3], [1, H * WP]]),
                        bass.AP(h2_t[s].tensor, 64 * H2SZ + ky * WP,
                                [[H2SZ, 1], [1, 3], [1, H * WP]]))

            def emit_B_tile(bi, t):
                s = bi % 2
                pdm = pdmp.tile([128, NT], F32, tag="pdm", name="pdm")
                pdms[s][t] = pdm
                nc.tensor.matmul(pdm[:], dilw[:], m9_t[s][:, NT * t:NT * (t + 1)],
                                 start=True, stop=True)
                # h1 *= (dilated mask): min(h1, BIG*dilsum)
                nc.vector.tensor_tensor(
                    h1_ap(s, t), h1_ap(s, t),
                    pdm[0:WID, :].rearrange("p (r w) -> p r w", w=WP)[:, :, 1:57],
                    mybir.AluOpType.min)
                # duplicate rows shifted up one padded row into parts 64-127
                dma(h1_t[s][64:128, 1 + 8 * t * WP: 1 + 8 * t * WP + NT],
                    h1_t[s][0:WID, 1 + (8 * t + 1) * WP: 1 + (8 * t + 1) * WP + NT])

            def emit_C_tile(bi, t):
                s = bi % 2
                p2 = p2p.tile([WID, NT], F32, tag="p2", name="p2")
                for kx in range(3):
                    nc.tensor.matmul(
                        p2[:], w2a[:, WID * kx: WID * (kx + 1)],
                        h1_t[s][0:128, 1 + 8 * t * WP + kx - 1:
                                1 + 8 * t * WP + kx - 1 + NT],
                        start=(kx == 0), stop=False)
                for kx in range(3):
                    nc.tensor.matmul(
                        p2[:], w2b[:, WID * kx: WID * (kx + 1)],
                        h1_t[s][0:WID, 1 + (8 * t + 2) * WP + kx - 1:
                                1 + (8 * t + 2) * WP + kx - 1 + NT],
                        start=False, stop=(kx == 2))
                tmp = tmpp.tile([WID, NT], DT_C3, tag="tmp", name="tmp")
                nc.scalar.activation(tmp[:], p2[:],
                                     mybir.ActivationFunctionType.Relu,
                                     bias=b2s[:, 0:1])
                nc.vector.tensor_tensor(
                    h2_int_ap(s, t),
                    tmp[:].rearrange("p (r w) -> p r w", w=WP)[:, :, 1:57],
                    pdms[s][t][64:128, :].rearrange("p (r w) -> p r w", w=WP)[:, :, 1:57],
                    mybir.AluOpType.min)

            def emit_D_tile(bi, t):
                b, s, s3 = bi % BL, bi % 2, bi % 3
                for c in (0, 1):
                    p3 = p3p.tile([128, NT], F32, tag="p3", name="p3")
                    nc.tensor.matmul(
                        p3[:], w3a[:, 128 * c:128 * (c + 1)],
                        h2_t[s][:, 1 + (8 * t + 1) * WP:
                                1 + (8 * t + 1) * WP + NT],
                        start=True, stop=True)
                    ob = outp.tile([128, NC1], F32, tag="ob", name="ob")
                    nc.vector.tensor_tensor(
                        ob[:].rearrange("p (r w) -> p r w", w=W),
                        p3[:].rearrange("p (r w) -> p r w", w=WP)[:, :, 1:57],
                        x_t[s3][c][:, 1 + NC1 * t: 1 + NC1 * (t + 1)]
                        .rearrange("p (r w) -> p r w", w=W),
                        mybir.AluOpType.add)
                    nc.scalar.activation(
                        ob[:], ob[:], mybir.ActivationFunctionType.Relu)
                    # store from the ACT hwdge queue (same engine that
                    # produced ob: no cross-engine hop, no SP blocking)
                    nc.scalar.dma_start(
                        out_d[b, 128 * c:128 * (c + 1)]
                        .rearrange("c h w -> c (h w)")[:, 8 * t * W:
                                                       8 * t * W + NC1],
                        ob[:])

            # Software-pipelined emission: stages C+D of image i-1 interleave
            # with stages A+B of image i, so the PE queue always holds ready
            # work while cross-engine epilogues (relu/min/gather/dup) chase.
            # C(i-1) opens each iteration: all its inputs are ready.
            for i in range(N + 1):
                if i == 0:
                    emit_load(0)
                if i + 1 < N:
                    emit_load(i + 1)  # prefetch next image's x
                for t in range(NTILES):
                    if i > 0:
                        emit_C_tile(i - 1, t)
                for t in range(NTILES):
                    if i < N:
                        emit_A_tile(i, t)
                if i < N:
                    emit_gather(i)
                # two D-tiles up front fill the is_ge->gather->dilate chase
                # with ready PE work; the rest interleave with the dilates.
                if i > 0:
                    emit_D_tile(i - 1, 0)
                    emit_D_tile(i - 1, 1)
                for t in range(NTILES):
                    if i < N:
                        emit_B_tile(i, t)
                    if i > 0 and t + 2 < NTILES:
                        emit_D_tile(i - 1, t + 2)

    nc.compile()
    return nc


def _fold_params(inputs, mode: str = "bf16"):
    import ml_dtypes
    fast = ml_dtypes.bfloat16 if mode == "bf16" else np.float32
    f = np.float32
    g1, b1, m1, v1 = (inputs[k].astype(f) for k in ("bn1_g", "bn1_b", "bn1_m", "bn1_v"))
    g2, b2, m2, v2 = (inputs[k].astype(f) for k in ("bn2_g", "bn2_b", "bn2_m", "bn2_v"))
    g3, b3, m3, v3 = (inputs[k].astype(f) for k in ("bn3_g", "bn3_b", "bn3_m", "bn3_v"))
    s1 = g1 / np.sqrt(v1 + EPS)
    s2 = g2 / np.sqrt(v2 + EPS)
    s3 = g3 / np.sqrt(v3 + EPS)

    w1 = inputs["conv1_w"].astype(f)[:, :, 0, 0]          # [64, 256]
    mw = inputs["mask_w"].astype(f)[:, :, 0, 0]           # [1, 256]
    w1aug = np.zeros((CIN, WID + 1), f)
    w1aug[:, :WID] = (w1 * s1[:, None]).T
    w1aug[:, WID] = mw[0]
    b1aug = np.zeros((WID + 1, 1), f)
    b1aug[:WID, 0] = b1 - m1 * s1
    b1aug[WID, 0] = float(inputs["mask_b"][0])

    w2 = inputs["conv2_w"].astype(f)                      # [64, 64, 3, 3]
    w2s = w2 * s2[:, None, None, None]
    w2k01 = np.zeros((128, 3 * WID), f)
    w2k2 = np.zeros((WID, 3 * WID), f)
    for kx in range(3):
        w2k01[0:WID, WID * kx:WID * (kx + 1)] = w2s[:, :, 0, kx].T
        w2k01[WID:128, WID * kx:WID * (kx + 1)] = w2s[:, :, 1, kx].T
        w2k2[:, WID * kx:WID * (kx + 1)] = w2s[:, :, 2, kx].T
    b2s_ = (b2 - m2 * s2).reshape(WID, 1)

    w3 = inputs["conv3_w"].astype(f)[:, :, 0, 0]          # [256, 64]
    w3aug = np.zeros((WID + 1, CIN), f)
    w3aug[:WID, :] = (w3 * s3[:, None]).T
    w3aug[WID, :] = b3 - m3 * s3

    dilw = np.zeros((9, 128), f)
    dilw[:, :WID] = BIG
    dilw[4, WID:128] = BIG

    return {
        "w1aug": w1aug, "b1aug": b1aug,
        "w2k01": w2k01.astype(fast), "w2k2": w2k2.astype(fast),
        "b2s": b2s_, "w3aug": w3aug.astype(fast), "dilw": dilw.astype(fast),
    }, float(inputs["mask_b"][0])


_NC_CACHE = {}


def build_program(mask_b_val: float, nreps: int = 1, mode: str = "bf16",
                  debug_soft: bool = False,
                  f32r_stages: frozenset = frozenset({"dil", "c2", "c3", "id"})):
    key = (mask_b_val, nreps, mode, debug_soft, f32r_stages)
    if key not in _NC_CACHE:
        _NC_CACHE[key] = _build_nc(mask_b_val, nreps, mode, debug_soft, f32r_stages)
    return _NC_CACHE[key]


def make_in_maps(inputs, mode: str = "bf16"):
    params, mask_b_val = _fold_params(inputs, mode)
    x = np.ascontiguousarray(inputs["x"], dtype=np.float32)
    in_maps = []
    for i in range(NCORES):
        m = dict(params)
        m["x"] = x[BL * i: BL * (i + 1)]
        in_maps.append(m)
    return in_maps, mask_b_val


def kernel(**inputs) -> np.ndarray:
    from concourse.bass_utils import run_bass_kernel_spmd

    in_maps, mask_b_val = make_in_maps(inputs)
    nc = build_program(mask_b_val)
    res = run_bass_kernel_spmd(nc, in_maps, list(range(NCORES)))
    out = np.concatenate([res.results[i]["out"] for i in range(NCORES)], axis=0)
    return out.astype(np.float32)


# revision 51
# speedup vs baseline: 1.2075x; 1.2075x over previous
"""Trainium2 Bass kernel for the masked-bottleneck block (topk_masking).

Full inputs in, full outputs out. Internally shards batch (32) across 8
NeuronCores (4 images each); all parameters replicated.

Per-core layout: channels on partitions, pixels on the free dim. Each 56x56
image is processed in 7 row-tiles of 8 rows; conv2's 3x3 halo is handled by
keeping h1 in a zero-padded 58x58 layout, with a +1-row-shifted duplicate in
partitions 64..127 so two of the three ky taps share one K=128 matmul.
The mask conv rides along as a 65th conv1 output row (conv1 in exact fp32 so
the sign thresholds match the fp32 reference bit-for-bit; min |soft| is
5.8e-6, so any reduced-precision conv1 flips mask bits). Dilation is a single
K=9 matmul over 9 shifted copies of the binary mask, which simultaneously
broadcasts the (scaled) mask across partitions; masking is a vector `min`
against BIG*mask. dilate/conv2/conv3 run in bf16 (1 cyc/col on the PE, half
the SBUF/DMA bytes). f32r is NOT used: on this toolchain mixing f32r matmuls
into the program degrades the exact fp32 conv1 to f32r-level error (~5e-4),
flipping mask bits.

Structural points vs the naive version: x is triple-buffered and loaded on
the Pool SWDGE queue (never queues behind stores); the mask9 gather is 3
DMAs (one per ky covering its 3 kx shifts) instead of 9; output stores issue
from the ACT hwdge queue right after the relu that produces them.
"""

import numpy as np

import concourse.bass as bass
import concourse.tile as tile
from concourse import bacc, mybir

EPS = 1e-5
BIG = 1e30

B, CIN, H, W = 32, 256, 56, 56
NCORES = 8
BL = B // NCORES          # images per core
WID = 64                  # bottleneck width
WP = W + 2                # padded row width  (58)
NPIX = H * W              # 3136
NPAD = WP * WP            # 3364
ROWS = 8                  # output rows per tile
NT = ROWS * WP            # 464 matmul free size per tile
NC1 = ROWS * W            # 448 conv1 tile size
NTILES = H // ROWS        # 7

# sbuf buffer geometries (elements per partition)
XSZ = 1 + NPIX + 3        # x: slack elem 0, data at 1..3136, tail slack
H1SZ = 1 + NPAD + 3       # h1: slack elem 0, padded image at 1..3364
H2SZ = 1 + NPAD + 1       # h2 (65 rows: 0-63 h2m, 64 mask01)
M9SZ = H * WP             # mask9: rows p=0..55, cols q=0..57 (3248)

F32 = mybir.dt.float32


def _build_nc(mask_b_val: float, nreps: int = 1, mode: str = "bf16",
              debug_soft: bool = False,
              f32r_stages: frozenset = frozenset({"dil", "c2", "c3", "id"})):
    if mode == "f32":
        f32r_stages = frozenset()
    fast_dt = mybir.dt.bfloat16 if mode == "bf16" else mybir.dt.float32r
    def dt_of(stage):
        return fast_dt if stage in f32r_stages else mybir.dt.float32
    DT_DIL, DT_C2, DT_C3, DT_ID = (dt_of(s) for s in ("dil", "c2", "c3", "id"))
    nc = bacc.Bacc("TRN2", target_bir_lowering=False, debug=False)

    x_d = nc.declare_dram_parameter("x", [BL, CIN, H, W], F32, isOutput=False)
    w1_d = nc.declare_dram_parameter("w1aug", [CIN, WID + 1], F32, isOutput=False)
    b1_d = nc.declare_dram_parameter("b1aug", [WID + 1, 1], F32, isOutput=False)
    w2a_d = nc.declare_dram_parameter("w2k01", [128, 3 * WID], DT_C2, isOutput=False)
    w2b_d = nc.declare_dram_parameter("w2k2", [WID, 3 * WID], DT_C2, isOutput=False)
    b2_d = nc.declare_dram_parameter("b2s", [WID, 1], F32, isOutput=False)
    w3_d = nc.declare_dram_parameter("w3aug", [WID + 1, CIN], DT_C3, isOutput=False)
    dil_d = nc.declare_dram_parameter("dilw", [9, 128], DT_DIL, isOutput=False)
    out_d = nc.declare_dram_parameter("out", [BL, CIN, H, W], F32, isOutput=True)
    soft_d = (nc.declare_dram_parameter("softdbg", [BL, NPIX], F32, isOutput=True)
              if debug_soft else None)

    with tile.TileContext(nc) as tc:
        with (
            tc.tile_pool(name="consts", bufs=1) as cpool,
            tc.tile_pool(name="bigbufs", bufs=1) as bigp,
            tc.tile_pool(name="outbuf", bufs=4) as outp,
            tc.tile_pool(name="tmp2", bufs=2) as tmpp,
            tc.tile_pool(name="p1", bufs=2, space="PSUM") as p1p,
            tc.tile_pool(name="pdm", bufs=2, space="PSUM") as pdmp,
            tc.tile_pool(name="p2", bufs=2, space="PSUM") as p2p,
            tc.tile_pool(name="p3", bufs=2, space="PSUM") as p3p,
        ):
            dma = nc.sync.dma_start

            # ---- constants -------------------------------------------------
            w1_lo = cpool.tile([128, WID + 1], F32, tag="w1lo")
            w1_hi = cpool.tile([128, WID + 1], F32, tag="w1hi")
            dma(w1_lo[:], w1_d[0:128, :])
            dma(w1_hi[:], w1_d[128:256, :])
            w2a = cpool.tile([128, 3 * WID], DT_C2, tag="w2a")
            dma(w2a[:], w2a_d[:])
            w2b = cpool.tile([WID, 3 * WID], DT_C2, tag="w2b")
            dma(w2b[:], w2b_d[:])
            w3a = cpool.tile([WID + 1, CIN], DT_C3, tag="w3a")
            dma(w3a[:], w3_d[:])
            dilw = cpool.tile([9, 128], DT_DIL, tag="dilw")
            dma(dilw[:], dil_d[:])
            b1s = cpool.tile([WID + 1, 1], F32, tag="b1s")
            dma(b1s[:], b1_d[:])
            b2s = cpool.tile([WID, 1], F32, tag="b2s")
            dma(b2s[:], b2_d[:])

            # ---- persistent multi-buffered image buffers -------------------
            # x is triple-buffered: the load of image i+1 then only waits on
            # image i-2's readers, so it streams well ahead of conv1(i+1).
            x_t = [[bigp.tile([128, XSZ], F32, tag=f"x{s}c{c}", name=f"x{s}c{c}")
                    for c in (0, 1)] for s in (0, 1, 2)]
            h1_t = [bigp.tile([128, H1SZ], DT_C2, tag=f"h1{s}", name=f"h1{s}")
                    for s in (0, 1)]
            h2_t = [bigp.tile([WID + 1, H2SZ], DT_C3, tag=f"h2{s}", name=f"h2{s}")
                    for s in (0, 1)]
            m9_t = [bigp.tile([9, M9SZ], DT_DIL, tag=f"m9{s}", name=f"m9{s}")
                    for s in (0, 1)]

            # one-time zero of padding (interior is rewritten every image,
            # pads/slack must be exactly 0 for conv halos and mask taps)
            for s in (0, 1):
                for tl, sz, np_ in ((h1_t[s], H1SZ, 128), (h2_t[s], H2SZ, WID + 1)):
                    nc.vector.memset(tl[:, 0:1 + WP], 0.0)
                    nc.vector.memset(
                        bass.AP(tl.tensor, 1 + W + 1,
                                [[sz, np_], [WP, WP - 1], [1, 2]]),
                        0.0)
                    nc.vector.memset(tl[:, 1 + (WP - 1) * WP: sz], 0.0)
            for s in (0, 1, 2):
                for c in (0, 1):
                    nc.vector.memset(x_t[s][c][:, 0:1], 0.0)
                    nc.vector.memset(x_t[s][c][:, 1 + NPIX: XSZ], 0.0)

            def h1_ap(s, t):
                # interior rows 8t+1..8t+8, cols 1..56 of padded h1 (strided)
                return bass.AP(h1_t[s].tensor, 1 + (8 * t + 1) * WP + 1,
                               [[H1SZ, WID], [WP, ROWS], [1, W]])

            def h2_row64_ap(s, t):
                return bass.AP(h2_t[s].tensor, 64 * H2SZ + 1 + (8 * t + 1) * WP + 1,
                               [[H2SZ, 1], [WP, ROWS], [1, W]])

            def h2_int_ap(s, t):
                return bass.AP(h2_t[s].tensor, 1 + (8 * t + 1) * WP + 1,
                               [[H2SZ, WID], [WP, ROWS], [1, W]])

            N = BL * nreps

            def emit_load(j):
                # on the Pool SWDGE queue: never queues behind stores/gathers
                for c in (0, 1):
                    nc.gpsimd.dma_start(
                        bass.AP(x_t[j % 3][c].tensor, 1, [[XSZ, 128], [1, NPIX]]),
                        x_d[j % BL, 128 * c:128 * (c + 1)]
                        .rearrange("c h w -> c (h w)"))

            pdms = {0: [None] * NTILES, 1: [None] * NTILES}

            def emit_A_tile(bi, t):
                b, s, s3 = bi % BL, bi % 2, bi % 3
                p1 = p1p.tile([WID + 1, NC1], F32, tag="p1", name="p1")
                nc.tensor.matmul(p1[:], w1_lo[:],
                                 x_t[s3][0][:, 1 + NC1 * t: 1 + NC1 * (t + 1)],
                                 start=True, stop=False)
                nc.tensor.matmul(p1[:], w1_hi[:],
                                 x_t[s3][1][:, 1 + NC1 * t: 1 + NC1 * (t + 1)],
                                 start=False, stop=True)
                # h1 = relu(conv1 + b1) into padded layout
                nc.scalar.activation(
                    h1_ap(s, t),
                    p1[0:WID, :].rearrange("p (r w) -> p r w", w=W),
                    mybir.ActivationFunctionType.Relu, bias=b1s[:WID, 0:1])
                if soft_d is not None:
                    sb_dbg = outp.tile([1, NC1], F32, tag="sdbg", name="sdbg")
                    nc.vector.tensor_copy(sb_dbg[:], p1[WID:WID + 1, :])
                    dma(soft_d[b, NC1 * t: NC1 * (t + 1)].unsqueeze(0), sb_dbg[:])
                # mask01 = (soft + mask_b >= 0) into h2 row 64
                nc.vector.tensor_scalar(
                    h2_row64_ap(s, t),
                    p1[WID:WID + 1, :].rearrange("p (r w) -> p r w", w=W),
                    -mask_b_val, None, mybir.AluOpType.is_ge)

            def emit_gather(bi):
                # mask9 gather: 9 shifted copies of mask01 in 3 DMAs; one DMA
                # per ky covers its 3 kx shifts (dst partitions 3ky..3ky+2)
                s = bi % 2
                for ky in range(3):
                    dma(bass.AP(m9_t[s].tensor, 3 * ky * M9SZ,
                                [[M9SZ, 3], [1, H * WP]]),
                        bass.AP(h2_t[s].tensor, 64 * H2SZ + ky * WP,
                                [[H2SZ, 1], [1, 3], [1, H * WP]]))

            def emit_B_tile(bi, t):
                s = bi % 2
                pdm = pdmp.tile([128, NT], F32, tag="pdm", name="pdm")
                pdms[s][t] = pdm
                nc.tensor.matmul(pdm[:], dilw[:], m9_t[s][:, NT * t:NT * (t + 1)],
                                 start=True, stop=True)
                # h1 *= (dilated mask): min(h1, BIG*dilsum)
                nc.vector.tensor_tensor(
                    h1_ap(s, t), h1_ap(s, t),
                    pdm[0:WID, :].rearrange("p (r w) -> p r w", w=WP)[:, :, 1:57],
                    mybir.AluOpType.min)
                # duplicate rows shifted up one padded row into parts 64-127
                dma(h1_t[s][64:128, 1 + 8 * t * WP: 1 + 8 * t * WP + NT],
                    h1_t[s][0:WID, 1 + (8 * t + 1) * WP: 1 + (8 * t + 1) * WP + NT])

            def emit_C_tile(bi, t):
                s = bi % 2
                p2 = p2p.tile([WID, NT], F32, tag="p2", name="p2")
                for kx in range(3):
                    nc.tensor.matmul(
                        p2[:], w2a[:, WID * kx: WID * (kx + 1)],
                        h1_t[s][0:128, 1 + 8 * t * WP + kx - 1:
                                1 + 8 * t * WP + kx - 1 + NT],
                        start=(kx == 0), stop=False)
                for kx in range(3):
                    nc.tensor.matmul(
                        p2[:], w2b[:, WID * kx: WID * (kx + 1)],
                        h1_t[s][0:WID, 1 + (8 * t + 2) * WP + kx - 1:
                                1 + (8 * t + 2) * WP + kx - 1 + NT],
                        start=False, stop=(kx == 2))
                tmp = tmpp.tile([WID, NT], DT_C3, tag="tmp", name="tmp")
                nc.scalar.activation(tmp[:], p2[:],
                                     mybir.ActivationFunctionType.Relu,
                                     bias=b2s[:, 0:1])
                nc.vector.tensor_tensor(
                    h2_int_ap(s, t),
                    tmp[:].rearrange("p (r w) -> p r w", w=WP)[:, :, 1:57],
                    pdms[s][t][64:128, :].rearrange("p (r w) -> p r w", w=WP)[:, :, 1:57],
                    mybir.AluOpType.min)

            def emit_D_tile(bi, t):
                b, s, s3 = bi % BL, bi % 2, bi % 3
                for c in (0, 1):
                    p3 = p3p.tile([128, NT], F32, tag="p3", name="p3")
                    nc.tensor.matmul(
                        p3[:], w3a[:, 128 * c:128 * (c + 1)],
                        h2_t[s][:, 1 + (8 * t + 1) * WP:
                                1 + (8 * t + 1) * WP + NT],
                        start=True, stop=True)
                    ob = outp.tile([128, NC1], F32, tag="ob", name="ob")
                    nc.vector.tensor_tensor(
                        ob[:].rearrange("p (r w) -> p r w", w=W),
                        p3[:].rearrange("p (r w) -> p r w", w=WP)[:, :, 1:57],
                        x_t[s3][c][:, 1 + NC1 * t: 1 + NC1 * (t + 1)]
                        .rearrange("p (r w) -> p r w", w=W),
                        mybir.AluOpType.add)
                    nc.scalar.activation(
                        ob[:], ob[:], mybir.ActivationFunctionType.Relu)
                    # store from the ACT hwdge queue (same engine that
                    # produced ob: no cross-engine hop, no SP blocking)
                    nc.scalar.dma_start(
                        out_d[b, 128 * c:128 * (c + 1)]
                        .rearrange("c h w -> c (h w)")[:, 8 * t * W:
                                                       8 * t * W + NC1],
                        ob[:])

            # Software-pipelined emission: stages C+D of image i-1 interleave
            # with stages A+B of image i, so the PE queue always holds ready
            # work while cross-engine epilogues (relu/min/gather/dup) chase.
            # C(i-1) opens each iteration: all its inputs are ready.
            for i in range(N + 1):
                if i == 0:
                    emit_load(0)
                if i + 1 < N:
                    emit_load(i + 1)  # prefetch next image's x
                for t in range(NTILES):
                    if i > 0:
                        emit_C_tile(i - 1, t)
                for t in range(NTILES):
                    if i < N:
                        emit_A_tile(i, t)
                if i < N:
                    emit_gather(i)
                for t in range(NTILES):
                    if i < N:
                        emit_B_tile(i, t)
                    if i > 0:
                        emit_D_tile(i - 1, t)

    nc.compile()
    return nc


def _fold_params(inputs, mode: str = "bf16"):
    import ml_dtypes
    fast = ml_dtypes.bfloat16 if mode == "bf16" else np.float32
    f = np.float32
    g1, b1, m1, v1 = (inputs[k].astype(f) for k in ("bn1_g", "bn1_b", "bn1_m", "bn1_v"))
    g2, b2, m2, v2 = (inputs[k].astype(f) for k in ("bn2_g", "bn2_b", "bn2_m", "bn2_v"))
    g3, b3, m3, v3 = (inputs[k].astype(f) for k in ("bn3_g", "bn3_b", "bn3_m", "bn3_v"))
    s1 = g1 / np.sqrt(v1 + EPS)
    s2 = g2 / np.sqrt(v2 + EPS)
    s3 = g3 / np.sqrt(v3 + EPS)

    w1 = inputs["conv1_w"].astype(f)[:, :, 0, 0]          # [64, 256]
    mw = inputs["mask_w"].astype(f)[:, :, 0, 0]           # [1, 256]
    w1aug = np.zeros((CIN, WID + 1), f)
    w1aug[:, :WID] = (w1 * s1[:, None]).T
    w1aug[:, WID] = mw[0]
    b1aug = np.zeros((WID + 1, 1), f)
    b1aug[:WID, 0] = b1 - m1 * s1
    b1aug[WID, 0] = float(inputs["mask_b"][0])

    w2 = inputs["conv2_w"].astype(f)                      # [64, 64, 3, 3]
    w2s = w2 * s2[:, None, None, None]
    w2k01 = np.zeros((128, 3 * WID), f)
    w2k2 = np.zeros((WID, 3 * WID), f)
    for kx in range(3):
        w2k01[0:WID, WID * kx:WID * (kx + 1)] = w2s[:, :, 0, kx].T
        w2k01[WID:128, WID * kx:WID * (kx + 1)] = w2s[:, :, 1, kx].T
        w2k2[:, WID * kx:WID * (kx + 1)] = w2s[:, :, 2, kx].T
    b2s_ = (b2 - m2 * s2).reshape(WID, 1)

    w3 = inputs["conv3_w"].astype(f)[:, :, 0, 0]          # [256, 64]
    w3aug = np.zeros((WID + 1, CIN), f)
    w3aug[:WID, :] = (w3 * s3[:, None]).T
    w3aug[WID, :] = b3 - m3 * s3

    dilw = np.zeros((9, 128), f)
    dilw[:, :WID] = BIG
    dilw[4, WID:128] = BIG

    return {
        "w1aug": w1aug, "b1aug": b1aug,
        "w2k01": w2k01.astype(fast), "w2k2": w2k2.astype(fast),
        "b2s": b2s_, "w3aug": w3aug.astype(fast), "dilw": dilw.astype(fast),
    }, float(inputs["mask_b"][0])


_NC_CACHE = {}


def build_program(mask_b_val: float, nreps: int = 1, mode: str = "bf16",
                  debug_soft: bool = False,
                  f32r_stages: frozenset = frozenset({"dil", "c2", "c3", "id"})):
    key = (mask_b_val, nreps, mode, debug_soft, f32r_stages)
    if key not in _NC_CACHE:
        _NC_CACHE[key] = _build_nc(mask_b_val, nreps, mode, debug_soft, f32r_stages)
    return _NC_CACHE[key]


def make_in_maps(inputs, mode: str = "bf16"):
    params, mask_b_val = _fold_params(inputs, mode)
    x = np.ascontiguousarray(inputs["x"], dtype=np.float32)
    in_maps = []
    for i in range(NCORES):
        m = dict(params)
        m["x"] = x[BL * i: BL * (i + 1)]
        in_maps.append(m)
    return in_maps, mask_b_val


def kernel(**inputs) -> np.ndarray:
    from concourse.bass_utils import run_bass_kernel_spmd

    in_maps, mask_b_val = make_in_maps(inputs)
    nc = build_program(mask_b_val)
    res = run_bass_kernel_spmd(nc, in_maps, list(range(NCORES)))
    out = np.concatenate([res.results[i]["out"] for i in range(NCORES)], axis=0)
    return out.astype(np.float32)
